# revision 30
# baseline (speedup 1.0000x reference)
"""TRN2 Bass kernel for nn_CustomBlock (cosine-normalized channel attention).

Per group n (8 groups -> 8 NeuronCores, pure data parallel):
  K = Wk @ X + Wk0;  Q = Wq @ X + Wq0            (X: [C,B])
  S[i,j] = sum_b Q[i,b] K[j,b]
  Y = S / sqrt(max(|Q_i|^2,eps') * max(|K_j|^2,eps'))
  SM = softmax over i (per column j); Z[j,b] = sum_i SM[i,j] X[i,b]

Single-core program, bf16 matmul inputs / f32 PSUM accumulation:
  X resident in SBUF (bf16) for the whole kernel -- read from HBM once;
  serves phase-1/2 lhsT tiles and phase-3 rhs directly.
  phase 1: KT[b,j] (SBUF-resident bf16) via matmuls with X tiles as
           stationary lhsT; row norms rk accumulated on the fly.
  phase 2 (per 512-wide i-slice): Q-slice computed on the fly (kept in
           SBUF, never spilled), then S panels = Q_panel^T @ KT;
           E = exp(rq_i * (S * rk_j)) -> DRAM bf16; E column sums
           accumulated by DVE into PSUM.
  phase 3: Z row-panels = E_colpanel^T @ X, scaled by 1/colsum per
           partition (colsum transposed to [128,CT] via tiny matmuls).
"""

import os
import sys
import time

import numpy as np

try:
    import concourse.bass as bass  # noqa: F401
except ImportError:
    for _p in (
        "/opt/trn_rl_repo",
        "/opt/pypackages",
        "/root/.axon_site/_ro/trn_rl_repo",
        "/root/.axon_site/_ro/pypackages",
    ):
        if _p not in sys.path:
            sys.path.append(_p)

import concourse.bacc as bacc
import concourse.mybir as mybir
import concourse.tile as tile

import ml_dtypes

BF16NP = ml_dtypes.bfloat16

P = 128
F32 = mybir.dt.float32
F32R = mybir.dt.float32r
BF16 = mybir.dt.bfloat16
AF = mybir.ActivationFunctionType
OP = mybir.AluOpType

N_CORES = 8
FULL_C = 2048
FULL_B = 2048

LAST_EXEC_NS = None
LAST_TRACE = None
TRACE = bool(os.environ.get("KERNEL_TRACE"))


def _install_ntff_shim():
    """Register antenv.axon_hooks (absent in this image) so
    run_bass_kernel_spmd(trace=True) can NTFF-profile via libaxon."""
    import types

    if "antenv.axon_hooks" in sys.modules:
        return
    try:
        import trn_agent_boot.trn_boot as tb

        hook = tb._ntff_profile_via_ctypes("/opt/axon/libaxon_pjrt.so")
    except Exception:
        hook = None
    m = types.ModuleType("antenv.axon_hooks")
    state = {"hook": hook}
    m.get_axon_ntff_profile_hook = lambda: state["hook"]
    m.set_axon_ntff_profile_hook = lambda h: state.update(hook=h)
    sys.modules["antenv.axon_hooks"] = m


def build_program(C, B, with_bias=True):
    """Build the single-core Bass program (same program for all cores)."""
    nc = bacc.Bacc("TRN2", target_bir_lowering=False, debug=False,
                   num_devices=N_CORES)

    CT = C // P           # channel tiles (i/j/c)
    BT = B // P           # b tiles
    S2 = 512
    NJ2 = C // S2         # 512-wide slices of C
    NB3 = B // S2         # 512-wide slices of B
    NPP = S2 // P         # 128-panels per slice

    x_d = nc.dram_tensor("x", [C, B], BF16, kind="ExternalInput").ap()
    wkt_d = nc.dram_tensor("wkt", [C + P, C], BF16,
                           kind="ExternalInput").ap()
    wqt_d = nc.dram_tensor("wqt", [C + P, C], BF16,
                           kind="ExternalInput").ap()
    z_d = nc.dram_tensor("z", [C, B], F32, kind="ExternalOutput").ap()

    with tile.TileContext(nc) as tc:
        with (
            tc.tile_pool(name="dram", bufs=1, space="DRAM") as dram,
            tc.tile_pool(name="xres", bufs=1) as xpool,
            tc.tile_pool(name="ktres", bufs=1) as ktpool,
            tc.tile_pool(name="w", bufs=2) as wpool,
            tc.tile_pool(name="qsl", bufs=1) as qpool,
            tc.tile_pool(name="stage", bufs=2) as stpool,
            tc.tile_pool(name="tmp", bufs=2) as tmppool,
            tc.tile_pool(name="zrow", bufs=2) as zpool,
            tc.tile_pool(name="ss", bufs=2) as sspool,
            tc.tile_pool(name="stat", bufs=1) as stat,
            tc.tile_pool(name="ps", bufs=8, space="PSUM") as ps,
        ):
            e_dm = dram.tile([CT, P, CT, P], BF16, tag="e")

            # constants / stats
            ones_row = stat.tile([1, P], BF16, tag="ones_row")
            ones_colr = stat.tile([P, 1], BF16, tag="ones_colr")
            ones_col = stat.tile([P, 1], F32, tag="ones_col")
            one1 = stat.tile([1, 1], BF16, tag="one1")
            rq = stat.tile([P, CT], F32, tag="rq")
            rcol = stat.tile([P, CT], F32, tag="rcol")
            rkrow = stat.tile([1, C], BF16, tag="rowbuf")
            RK = stat.tile([P, NJ2, S2], BF16, tag="RK")
            tmf = stat.tile([1, P], F32, tag="tmf")
            nc.vector.memset(ones_col[:], 1.0)
            nc.vector.memset(tmf[:], 1.0)
            with nc.allow_low_precision(reason="constants"):
                nc.scalar.copy(ones_row[:], tmf[:])
                nc.scalar.copy(ones_colr[:], ones_col[:])
                nc.scalar.copy(one1[:], tmf[0:1, 0:1])

            # resident tensors
            xres = xpool.tile([P, CT, B], BF16, tag="xres")
            kt = ktpool.tile([P, BT, C], BF16, tag="ktres")

            x_r = x_d.rearrange("(ct p) b -> p ct b", p=P)
            wkt_r = wkt_d.rearrange("(ct p) j -> p ct j", p=P)
            wqt_r = wqt_d.rearrange("(ct p) j -> p ct j", p=P)

            # Startup: interleave the first wk panel's stripes with the
            # b-major X sweeps so the first matmuls unblock after ~1 MiB
            # and the PE ramps while the rest streams in.
            wk0 = wpool.tile([P, CT + 1, S2], BF16, tag="w")
            js0 = slice(0, S2)
            stripes = [(0, 1), (1, 2), (2, 4)] + [
                (c0, min(c0 + 4, CT + 1))
                for c0 in range(4, CT + 1, 4)]
            bs0 = slice(0, S2)
            for c0, c1 in stripes:
                nc.sync.dma_start(wk0[:, c0:c1, :], wkt_r[:, c0:c1, js0])
                cx = min(c1, CT)
                if c0 < cx:
                    nc.sync.dma_start(xres[:, c0:cx, bs0],
                                      x_r[:, c0:cx, bs0])
            for bs in range(1, B // S2):
                bsl = slice(bs * S2, (bs + 1) * S2)
                for c0 in range(0, CT, 4):
                    nc.sync.dma_start(xres[:, c0 : c0 + 4, bsl],
                                      x_r[:, c0 : c0 + 4, bsl])

            # ---------------- phase 1: KT projection ----------------
            def emit_dk2(sl, ssk):
                # DK2 row for slice sl -> rkrow (emitted late so the PE
                # stream is not head-of-line blocked on the ACT/DVE tail)
                js_ = slice(sl * S2, (sl + 1) * S2)
                pr = ps.tile([1, S2], F32, tag="ps")
                nc.tensor.matmul(pr[:], ones_col[:], ssk[:],
                                 start=True, stop=True)
                r1 = tmppool.tile([1, S2], F32, tag="tmp2")
                nc.vector.tensor_scalar(r1[:], pr[:], 1e-6, None, OP.max)
                r2 = tmppool.tile([1, S2], F32, tag="tmp2")
                nc.scalar.sqrt(r2[:], r1[:])
                with nc.allow_low_precision(reason="bf16 rk"):
                    nc.vector.reciprocal(rkrow[0:1, js_], r2[:])

            pending_dk2 = None
            for sl in range(NJ2):
                js = slice(sl * S2, (sl + 1) * S2)
                if sl == 0:
                    wkp = wk0
                else:
                    wkp = wpool.tile([P, CT + 1, S2], BF16, tag="w")
                    nc.sync.dma_start(wkp[:], wkt_r[:, :, js])
                ssk = sspool.tile([P, S2], F32, tag="ss")
                for bt in range(BT):
                    xlhs = xres[:, :, bt * P : (bt + 1) * P]
                    psk = ps.tile([P, S2], F32, tag="ps")
                    for ct in range(CT):
                        nc.tensor.matmul(psk[:], xlhs[:, ct, :],
                                         wkp[:, ct, :],
                                         start=(ct == 0),
                                         stop=(not with_bias
                                               and ct == CT - 1))
                    if with_bias:
                        nc.tensor.matmul(psk[:], ones_row[:],
                                         wkp[0:1, CT, :],
                                         start=False, stop=True)
                    with nc.allow_low_precision(reason="bf16 KT"):
                        nc.scalar.copy(kt[:, bt, js], psk[:])
                    if bt == 0:
                        nc.scalar.square(ssk[:], psk[:])
                    else:
                        sq = tmppool.tile([P, S2], F32, tag="tmp")
                        nc.scalar.square(sq[:], psk[:])
                        nc.vector.tensor_tensor(ssk[:], ssk[:], sq[:],
                                                OP.add)
                    if bt == 1 and pending_dk2 is not None:
                        pending_dk2()
                        pending_dk2 = None
                pending_dk2 = (lambda sl=sl, ssk=ssk:
                               emit_dk2(sl, ssk))

            # -------- phase 2: Q slices, scores, exp, colsum --------
            crow = stat.tile([1, C], BF16, tag="rowbuf")
            cs = [
                ps.tile([P, S2], F32, tag="ps", name=f"cs{j}")
                for j in range(NJ2)
            ]
            def emit_colsum(jsl):
                # colsum finalize rides one S-group behind its final add
                accb = stpool.tile([P, S2], BF16, tag="stage")
                with nc.allow_low_precision(reason="colsum reduce"):
                    nc.scalar.copy(accb[:], cs[jsl][:])
                pcr = ps.tile([1, S2], F32, tag="ps")
                nc.tensor.matmul(pcr[:], ones_colr[:], accb[:],
                                 start=True, stop=True)
                with nc.allow_low_precision(reason="colsum f32r"):
                    nc.scalar.copy(
                        crow[0:1, jsl * S2 : (jsl + 1) * S2], pcr[:])

            pending_colsum = None

            def emit_dq2(isl, ssq):
                for k in range(NPP):
                    pq = ps.tile([P, 1], F32, tag="ps")
                    nc.tensor.matmul(pq[:],
                                     ssq[:, k * P : (k + 1) * P],
                                     ones_col[:], start=True, stop=True)
                    c1_ = tmppool.tile([P, 1], F32, tag="tmp")
                    nc.vector.tensor_scalar(c1_[:], pq[:], 1e-6, None,
                                            OP.max)
                    c2_ = tmppool.tile([P, 1], F32, tag="tmp")
                    nc.scalar.sqrt(c2_[:], c1_[:])
                    idx = isl * NPP + k
                    nc.vector.reciprocal(rq[:, idx : idx + 1], c2_[:])

            def emit_rk_broadcast():
                for jsl in range(NJ2):
                    js2 = slice(jsl * S2, (jsl + 1) * S2)
                    psb = ps.tile([P, S2], F32, tag="ps")
                    nc.tensor.matmul(psb[:], ones_row[:],
                                     rkrow[0:1, js2],
                                     start=True, stop=True)
                    with nc.allow_low_precision(reason="bf16 RK"):
                        nc.scalar.copy(RK[:, jsl, :], psb[:])

            for isl in range(NJ2):
                iss = slice(isl * S2, (isl + 1) * S2)
                wqp = wpool.tile([P, CT + 1, S2], BF16, tag="w")
                for c0, c1 in stripes:
                    nc.sync.dma_start(wqp[:, c0:c1, :],
                                      wqt_r[:, c0:c1, iss])
                qsl = qpool.tile([P, BT, S2], BF16, tag="qsl")
                ssq = sspool.tile([P, S2], F32, tag="ss")
                for bt in range(BT):
                    xlhs = xres[:, :, bt * P : (bt + 1) * P]
                    psq = ps.tile([P, S2], F32, tag="ps")
                    for ct in range(CT):
                        nc.tensor.matmul(psq[:], xlhs[:, ct, :],
                                         wqp[:, ct, :],
                                         start=(ct == 0),
                                         stop=(not with_bias
                                               and ct == CT - 1))
                    if with_bias:
                        nc.tensor.matmul(psq[:], ones_row[:],
                                         wqp[0:1, CT, :],
                                         start=False, stop=True)
                    with nc.allow_low_precision(reason="bf16 Q"):
                        nc.scalar.copy(qsl[:, bt, :], psq[:])
                    if bt == 0:
                        nc.scalar.square(ssq[:], psq[:])
                    else:
                        sq2 = tmppool.tile([P, S2], F32, tag="tmp")
                        nc.scalar.square(sq2[:], psq[:])
                        nc.vector.tensor_tensor(ssq[:], ssq[:], sq2[:],
                                                OP.add)
                    if bt == 1 and pending_dk2 is not None:
                        pending_dk2()
                        pending_dk2 = None
                # scores + exp + colsum for the panels of this slice
                for ipl in range(NPP):
                    ip = isl * NPP + ipl
                    qp = qsl[:, :, ipl * P : (ipl + 1) * P]
                    for jsl in range(NJ2):
                        js2 = slice(jsl * S2, (jsl + 1) * S2)
                        pss = ps.tile([P, S2], F32, tag="ps")
                        for bt in range(BT):
                            nc.tensor.matmul(
                                pss[:], qp[:, bt, :], kt[:, bt, js2],
                                start=(bt == 0), stop=(bt == BT - 1),
                            )
                        if ipl == 0 and jsl == 0:
                            # stat work rides behind the first S group
                            emit_dq2(isl, ssq)
                            if isl == 0:
                                emit_rk_broadcast()
                        tm = tmppool.tile([P, S2], F32, tag="tmp2")
                        nc.vector.tensor_tensor(tm[:], pss[:],
                                                RK[:, jsl, :], OP.mult)
                        et = stpool.tile([P, S2], BF16, tag="stage")
                        with nc.allow_low_precision(reason="bf16 E"):
                            nc.scalar.activation(et[:], tm[:], AF.Exp,
                                                 scale=rq[:, ip : ip + 1])
                        if ip == 0:
                            nc.vector.tensor_copy(cs[jsl][:], et[:])
                        else:
                            nc.vector.tensor_tensor(cs[jsl][:],
                                                    cs[jsl][:], et[:],
                                                    OP.add)
                        for k in range(NPP):
                            nc.sync.dma_start(
                                e_dm[jsl * NPP + k, :, ip, :],
                                et[:, k * P : (k + 1) * P],
                            )

            # ---------------- phase 3: Z = SM^T X ----------------
            # colsum -> rcol chain is emitted behind the first Z matmul
            # group so it never head-of-line blocks the PE stream.
            for jt in range(CT):
                ept = wpool.tile([P, CT, P], BF16, tag="w")
                nc.sync.dma_start(ept[:], e_dm[jt, :, :, :])
                for bsl in range(NB3):
                    bs2 = slice(bsl * S2, (bsl + 1) * S2)
                    psz = ps.tile([P, S2], F32, tag="ps")
                    for ic in range(CT):
                        nc.tensor.matmul(
                            psz[:], ept[:, ic, :], xres[:, ic, bs2],
                            start=(ic == 0), stop=(ic == CT - 1),
                        )
                    if jt == 0 and bsl == 0:
                        for jsl in range(NJ2):
                            emit_colsum(jsl)
                        pq2 = ps.tile([P, CT], F32, tag="ps")
                        for jt_ in range(CT):
                            nc.tensor.matmul(
                                pq2[:, jt_ : jt_ + 1],
                                crow[0:1, jt_ * P : (jt_ + 1) * P],
                                one1[:], start=True, stop=True)
                        nc.vector.reciprocal(rcol[:], pq2[:])
                    zt = zpool.tile([P, S2], F32, tag="zrow2")
                    nc.scalar.mul(zt[:], psz[:], rcol[:, jt : jt + 1])
                    nc.sync.dma_start(
                        z_d[jt * P : (jt + 1) * P, bs2], zt[:]
                    )

    nc.compile()
    return nc


def _host_prep(X, Wk, Wq, Wk0, Wq0, C):
    wkt = np.concatenate(
        [np.ascontiguousarray(Wk.T),
         Wk0.reshape(1, C),
         np.zeros((P - 1, C), np.float32)], axis=0
    ).astype(BF16NP)
    wqt = np.concatenate(
        [np.ascontiguousarray(Wq.T),
         Wq0.reshape(1, C),
         np.zeros((P - 1, C), np.float32)], axis=0
    ).astype(BF16NP)
    xb = np.ascontiguousarray(X).astype(BF16NP)
    return xb, wkt, wqt


_CACHE = {}


def kernel(X, Wk, Wq, Wk0, Wq0):
    global LAST_EXEC_NS, LAST_TRACE
    X = np.asarray(X, dtype=np.float32)
    Wk = np.asarray(Wk, dtype=np.float32)
    Wq = np.asarray(Wq, dtype=np.float32)
    Wk0 = np.asarray(Wk0, dtype=np.float32)
    Wq0 = np.asarray(Wq0, dtype=np.float32)
    N, C, B = X.shape
    assert N == N_CORES

    from concourse.bass_utils import run_bass_kernel_spmd

    with_bias = bool(np.any(Wk0) or np.any(Wq0))
    key = (C, B, with_bias)
    if key not in _CACHE:
        _CACHE[key] = build_program(C, B, with_bias)
    nc = _CACHE[key]

    xb, wkt, wqt = _host_prep(X, Wk, Wq, Wk0, Wq0, C)
    in_maps = [
        {"x": xb[n], "wkt": wkt, "wqt": wqt}
        for n in range(N)
    ]
    if TRACE:
        _install_ntff_shim()
    t0 = time.time()
    res = run_bass_kernel_spmd(
        nc, in_maps, core_ids=list(range(N_CORES)), trace=TRACE
    )
    wall_ns = int((time.time() - t0) * 1e9)
    LAST_EXEC_NS = (
        res.exec_time_ns if getattr(res, "exec_time_ns", None) else wall_ns
    )
    if getattr(res, "instructions_and_trace", None):
        LAST_TRACE = res.instructions_and_trace[1]
    out = np.stack([res.results[n]["z"] for n in range(N)], axis=0)
    return out.astype(np.float32)


if __name__ == "__main__":
    # small-scale self-test vs numpy
    C, B = 512, 512
    rng = np.random.default_rng(1)
    Xs = rng.standard_normal((N_CORES, C, B), dtype=np.float32)
    bound = float(np.sqrt(6.0 / (C + C)))
    Wks = rng.uniform(-bound, bound, (C, C)).astype(np.float32)
    Wqs = rng.uniform(-bound, bound, (C, C)).astype(np.float32)
    Wk0s = rng.standard_normal((C, 1)).astype(np.float32) * 0.01
    Wq0s = rng.standard_normal((C, 1)).astype(np.float32) * 0.01

    def ref(X, Wk, Wq, Wk0, Wq0):
        K = np.einsum("ij,njb->nib", Wk, X) + Wk0
        Q = np.einsum("ij,njb->nib", Wq, X) + Wq0
        DK2 = np.sum(K * K, axis=2)
        DQ2 = np.sum(Q * Q, axis=2)
        DQK = np.sqrt(np.maximum(DQ2[:, :, None] * DK2[:, None, :], 1e-12))
        Y = np.einsum("nib,njb->nij", Q, K) / DQK
        Y = Y - Y.max(axis=1, keepdims=True)
        E = np.exp(Y)
        SM = E / E.sum(axis=1, keepdims=True)
        return np.einsum("ncb,ncj->njb", X, SM)

    expected = ref(
        Xs.astype(np.float64), Wks.astype(np.float64),
        Wqs.astype(np.float64), Wk0s.astype(np.float64),
        Wq0s.astype(np.float64),
    )
    actual = kernel(Xs, Wks, Wqs, Wk0s, Wq0s)
    rel = np.linalg.norm(actual - expected) / np.linalg.norm(expected)
    print(f"small test relative error: {rel:.3e}")
    print(f"wall ns: {LAST_EXEC_NS}")


# revision 31
# speedup vs baseline: 1.0020x; 1.0020x over previous
"""TRN2 Bass kernel for nn_CustomBlock (cosine-normalized channel attention).

Per group n (8 groups -> 8 NeuronCores, pure data parallel):
  K = Wk @ X + Wk0;  Q = Wq @ X + Wq0            (X: [C,B])
  S[i,j] = sum_b Q[i,b] K[j,b]
  Y = S / sqrt(max(|Q_i|^2,eps') * max(|K_j|^2,eps'))
  SM = softmax over i (per column j); Z[j,b] = sum_i SM[i,j] X[i,b]

Single-core program, bf16 matmul inputs / f32 PSUM accumulation:
  X resident in SBUF (bf16) for the whole kernel -- read from HBM once;
  serves phase-1/2 lhsT tiles and phase-3 rhs directly.
  phase 1: KT[b,j] (SBUF-resident bf16) via matmuls with X tiles as
           stationary lhsT; row norms rk accumulated on the fly.
  phase 2 (per 512-wide i-slice): Q-slice computed on the fly (kept in
           SBUF, never spilled), then S panels = Q_panel^T @ KT;
           E = exp(rq_i * (S * rk_j)) -> DRAM bf16; E column sums
           accumulated by DVE into PSUM.
  phase 3: Z row-panels = E_colpanel^T @ X, scaled by 1/colsum per
           partition (colsum transposed to [128,CT] via tiny matmuls).
"""

import os
import sys
import time

import numpy as np

try:
    import concourse.bass as bass  # noqa: F401
except ImportError:
    for _p in (
        "/opt/trn_rl_repo",
        "/opt/pypackages",
        "/root/.axon_site/_ro/trn_rl_repo",
        "/root/.axon_site/_ro/pypackages",
    ):
        if _p not in sys.path:
            sys.path.append(_p)

import concourse.bacc as bacc
import concourse.mybir as mybir
import concourse.tile as tile
import concourse.bass_isa as bass_isa

import ml_dtypes

BF16NP = ml_dtypes.bfloat16

P = 128
F32 = mybir.dt.float32
F32R = mybir.dt.float32r
BF16 = mybir.dt.bfloat16
AF = mybir.ActivationFunctionType
OP = mybir.AluOpType

N_CORES = 8
FULL_C = 2048
FULL_B = 2048

LAST_EXEC_NS = None
LAST_TRACE = None
TRACE = bool(os.environ.get("KERNEL_TRACE"))


def _install_ntff_shim():
    """Register antenv.axon_hooks (absent in this image) so
    run_bass_kernel_spmd(trace=True) can NTFF-profile via libaxon."""
    import types

    if "antenv.axon_hooks" in sys.modules:
        return
    try:
        import trn_agent_boot.trn_boot as tb

        hook = tb._ntff_profile_via_ctypes("/opt/axon/libaxon_pjrt.so")
    except Exception:
        hook = None
    m = types.ModuleType("antenv.axon_hooks")
    state = {"hook": hook}
    m.get_axon_ntff_profile_hook = lambda: state["hook"]
    m.set_axon_ntff_profile_hook = lambda h: state.update(hook=h)
    sys.modules["antenv.axon_hooks"] = m


def build_program(C, B, with_bias=True):
    """Build the single-core Bass program (same program for all cores)."""
    nc = bacc.Bacc("TRN2", target_bir_lowering=False, debug=False,
                   num_devices=N_CORES)

    CT = C // P           # channel tiles (i/j/c)
    BT = B // P           # b tiles
    S2 = 512
    NJ2 = C // S2         # 512-wide slices of C
    NB3 = B // S2         # 512-wide slices of B
    NPP = S2 // P         # 128-panels per slice

    x_d = nc.dram_tensor("x", [C, B], BF16, kind="ExternalInput").ap()
    wkt_d = nc.dram_tensor("wkt", [C + P, C], BF16,
                           kind="ExternalInput").ap()
    wqt_d = nc.dram_tensor("wqt", [C + P, C], BF16,
                           kind="ExternalInput").ap()
    z_d = nc.dram_tensor("z", [C, B], F32, kind="ExternalOutput").ap()

    with tile.TileContext(nc) as tc:
        with (
            tc.tile_pool(name="dram", bufs=1, space="DRAM") as dram,
            tc.tile_pool(name="xres", bufs=1) as xpool,
            tc.tile_pool(name="ktres", bufs=1) as ktpool,
            tc.tile_pool(name="w", bufs=2) as wpool,
            tc.tile_pool(name="qsl", bufs=1) as qpool,
            tc.tile_pool(name="stage", bufs=2) as stpool,
            tc.tile_pool(name="tmp", bufs=2) as tmppool,
            tc.tile_pool(name="zrow", bufs=2) as zpool,
            tc.tile_pool(name="ss", bufs=2) as sspool,
            tc.tile_pool(name="stat", bufs=1) as stat,
            tc.tile_pool(name="ps", bufs=8, space="PSUM") as ps,
        ):
            e_dm = dram.tile([CT, P, CT, P], BF16, tag="e")

            # constants / stats
            ones_row = stat.tile([1, P], BF16, tag="ones_row")
            ones_colr = stat.tile([P, 1], BF16, tag="ones_colr")
            ones_col = stat.tile([P, 1], F32, tag="ones_col")
            one1 = stat.tile([1, 1], BF16, tag="one1")
            rq = stat.tile([P, CT], F32, tag="rq")
            rcol = stat.tile([P, CT], F32, tag="rcol")
            rkrow = stat.tile([1, C], BF16, tag="rowbuf")
            RK = stat.tile([P, NJ2, S2], BF16, tag="RK")
            tmf = stat.tile([1, P], F32, tag="tmf")
            nc.vector.memset(ones_col[:], 1.0)
            nc.vector.memset(tmf[:], 1.0)
            with nc.allow_low_precision(reason="constants"):
                nc.scalar.copy(ones_row[:], tmf[:])
                nc.scalar.copy(ones_colr[:], ones_col[:])
                nc.scalar.copy(one1[:], tmf[0:1, 0:1])

            # resident tensors
            xres = xpool.tile([P, CT, B], BF16, tag="xres")
            kt = ktpool.tile([P, BT, C], BF16, tag="ktres")

            x_r = x_d.rearrange("(ct p) b -> p ct b", p=P)
            wkt_r = wkt_d.rearrange("(ct p) j -> p ct j", p=P)
            wqt_r = wqt_d.rearrange("(ct p) j -> p ct j", p=P)

            # Startup: interleave the first wk panel's stripes with the
            # b-major X sweeps so the first matmuls unblock after ~1 MiB
            # and the PE ramps while the rest streams in.
            wk0 = wpool.tile([P, CT + 1, S2], BF16, tag="w")
            js0 = slice(0, S2)
            stripes = [(0, 1), (1, 2), (2, 4)] + [
                (c0, min(c0 + 4, CT + 1))
                for c0 in range(4, CT + 1, 4)]
            bs0 = slice(0, S2)
            for c0, c1 in stripes:
                nc.sync.dma_start(wk0[:, c0:c1, :], wkt_r[:, c0:c1, js0])
                cx = min(c1, CT)
                if c0 < cx:
                    nc.sync.dma_start(xres[:, c0:cx, bs0],
                                      x_r[:, c0:cx, bs0])
            for bs in range(1, B // S2):
                bsl = slice(bs * S2, (bs + 1) * S2)
                for c0 in range(0, CT, 4):
                    nc.sync.dma_start(xres[:, c0 : c0 + 4, bsl],
                                      x_r[:, c0 : c0 + 4, bsl])

            # ---------------- phase 1: KT projection ----------------
            def emit_dk2(sl, ssk):
                # DK2 row for slice sl -> rkrow (emitted late so the PE
                # stream is not head-of-line blocked on the ACT/DVE tail)
                js_ = slice(sl * S2, (sl + 1) * S2)
                ta = tmppool.tile([P, S2], F32, tag="tmp2")
                nc.gpsimd.partition_all_reduce(ta[:], ssk[:], P,
                                               bass_isa.ReduceOp.add)
                r1 = tmppool.tile([1, S2], F32, tag="tmp2")
                nc.vector.tensor_scalar(r1[:], ta[0:1, :], 1e-6, None,
                                        OP.max)
                r2 = tmppool.tile([1, S2], F32, tag="tmp2")
                nc.scalar.sqrt(r2[:], r1[:])
                with nc.allow_low_precision(reason="bf16 rk"):
                    nc.vector.reciprocal(rkrow[0:1, js_], r2[:])

            pending_dk2 = None
            for sl in range(NJ2):
                js = slice(sl * S2, (sl + 1) * S2)
                if sl == 0:
                    wkp = wk0
                else:
                    wkp = wpool.tile([P, CT + 1, S2], BF16, tag="w")
                    nc.sync.dma_start(wkp[:], wkt_r[:, :, js])
                ssk = sspool.tile([P, S2], F32, tag="ss")
                for bt in range(BT):
                    xlhs = xres[:, :, bt * P : (bt + 1) * P]
                    psk = ps.tile([P, S2], F32, tag="ps")
                    for ct in range(CT):
                        nc.tensor.matmul(psk[:], xlhs[:, ct, :],
                                         wkp[:, ct, :],
                                         start=(ct == 0),
                                         stop=(not with_bias
                                               and ct == CT - 1))
                    if with_bias:
                        nc.tensor.matmul(psk[:], ones_row[:],
                                         wkp[0:1, CT, :],
                                         start=False, stop=True)
                    with nc.allow_low_precision(reason="bf16 KT"):
                        nc.scalar.copy(kt[:, bt, js], psk[:])
                    if bt == 0:
                        nc.scalar.square(ssk[:], psk[:])
                    else:
                        sq = tmppool.tile([P, S2], F32, tag="tmp")
                        nc.scalar.square(sq[:], psk[:])
                        nc.vector.tensor_tensor(ssk[:], ssk[:], sq[:],
                                                OP.add)
                    if bt == 1 and pending_dk2 is not None:
                        pending_dk2()
                        pending_dk2 = None
                pending_dk2 = (lambda sl=sl, ssk=ssk:
                               emit_dk2(sl, ssk))

            # -------- phase 2: Q slices, scores, exp, colsum --------
            crow = stat.tile([1, C], BF16, tag="rowbuf")
            cs = [
                ps.tile([P, S2], F32, tag="ps", name=f"cs{j}")
                for j in range(NJ2)
            ]
            def emit_colsum(jsl):
                # colsum finalize rides one S-group behind its final add
                accb = stpool.tile([P, S2], BF16, tag="stage")
                with nc.allow_low_precision(reason="colsum reduce"):
                    nc.scalar.copy(accb[:], cs[jsl][:])
                ta2 = tmppool.tile([P, S2], F32, tag="tmp2")
                nc.gpsimd.partition_all_reduce(ta2[:], accb[:], P,
                                               bass_isa.ReduceOp.add)
                with nc.allow_low_precision(reason="colsum f32r"):
                    nc.scalar.copy(
                        crow[0:1, jsl * S2 : (jsl + 1) * S2],
                        ta2[0:1, :])

            pending_colsum = None

            def emit_dq2(isl, ssq):
                for k in range(NPP):
                    pq = ps.tile([P, 1], F32, tag="ps")
                    nc.tensor.matmul(pq[:],
                                     ssq[:, k * P : (k + 1) * P],
                                     ones_col[:], start=True, stop=True)
                    c1_ = tmppool.tile([P, 1], F32, tag="tmp")
                    nc.vector.tensor_scalar(c1_[:], pq[:], 1e-6, None,
                                            OP.max)
                    c2_ = tmppool.tile([P, 1], F32, tag="tmp")
                    nc.scalar.sqrt(c2_[:], c1_[:])
                    idx = isl * NPP + k
                    nc.vector.reciprocal(rq[:, idx : idx + 1], c2_[:])

            def emit_rk_broadcast():
                for jsl in range(NJ2):
                    js2 = slice(jsl * S2, (jsl + 1) * S2)
                    nc.gpsimd.partition_broadcast(RK[:, jsl, :],
                                                  rkrow[0:1, js2], P)

            for isl in range(NJ2):
                iss = slice(isl * S2, (isl + 1) * S2)
                wqp = wpool.tile([P, CT + 1, S2], BF16, tag="w")
                for c0, c1 in stripes:
                    nc.sync.dma_start(wqp[:, c0:c1, :],
                                      wqt_r[:, c0:c1, iss])
                qsl = qpool.tile([P, BT, S2], BF16, tag="qsl")
                ssq = sspool.tile([P, S2], F32, tag="ss")
                for bt in range(BT):
                    xlhs = xres[:, :, bt * P : (bt + 1) * P]
                    psq = ps.tile([P, S2], F32, tag="ps")
                    for ct in range(CT):
                        nc.tensor.matmul(psq[:], xlhs[:, ct, :],
                                         wqp[:, ct, :],
                                         start=(ct == 0),
                                         stop=(not with_bias
                                               and ct == CT - 1))
                    if with_bias:
                        nc.tensor.matmul(psq[:], ones_row[:],
                                         wqp[0:1, CT, :],
                                         start=False, stop=True)
                    with nc.allow_low_precision(reason="bf16 Q"):
                        nc.scalar.copy(qsl[:, bt, :], psq[:])
                    if bt == 0:
                        nc.scalar.square(ssq[:], psq[:])
                    else:
                        sq2 = tmppool.tile([P, S2], F32, tag="tmp")
                        nc.scalar.square(sq2[:], psq[:])
                        nc.vector.tensor_tensor(ssq[:], ssq[:], sq2[:],
                                                OP.add)
                    if bt == 1 and pending_dk2 is not None:
                        pending_dk2()
                        pending_dk2 = None
                # scores + exp + colsum for the panels of this slice
                for ipl in range(NPP):
                    ip = isl * NPP + ipl
                    qp = qsl[:, :, ipl * P : (ipl + 1) * P]
                    for jsl in range(NJ2):
                        js2 = slice(jsl * S2, (jsl + 1) * S2)
                        pss = ps.tile([P, S2], F32, tag="ps")
                        for bt in range(BT):
                            nc.tensor.matmul(
                                pss[:], qp[:, bt, :], kt[:, bt, js2],
                                start=(bt == 0), stop=(bt == BT - 1),
                            )
                        if ipl == 0 and jsl == 0:
                            # stat work rides behind the first S group
                            emit_dq2(isl, ssq)
                            if isl == 0:
                                emit_rk_broadcast()
                        tm = tmppool.tile([P, S2], F32, tag="tmp2")
                        nc.vector.tensor_tensor(tm[:], pss[:],
                                                RK[:, jsl, :], OP.mult)
                        et = stpool.tile([P, S2], BF16, tag="stage")
                        with nc.allow_low_precision(reason="bf16 E"):
                            nc.scalar.activation(et[:], tm[:], AF.Exp,
                                                 scale=rq[:, ip : ip + 1])
                        if ip == 0:
                            nc.vector.tensor_copy(cs[jsl][:], et[:])
                        else:
                            nc.vector.tensor_tensor(cs[jsl][:],
                                                    cs[jsl][:], et[:],
                                                    OP.add)
                        for k in range(NPP):
                            nc.sync.dma_start(
                                e_dm[jsl * NPP + k, :, ip, :],
                                et[:, k * P : (k + 1) * P],
                            )

            # ---------------- phase 3: Z = SM^T X ----------------
            # colsum -> rcol chain is emitted behind the first Z matmul
            # group so it never head-of-line blocks the PE stream.
            for jt in range(CT):
                ept = wpool.tile([P, CT, P], BF16, tag="w")
                nc.sync.dma_start(ept[:], e_dm[jt, :, :, :])
                for bsl in range(NB3):
                    bs2 = slice(bsl * S2, (bsl + 1) * S2)
                    psz = ps.tile([P, S2], F32, tag="ps")
                    for ic in range(CT):
                        nc.tensor.matmul(
                            psz[:], ept[:, ic, :], xres[:, ic, bs2],
                            start=(ic == 0), stop=(ic == CT - 1),
                        )
                    if jt == 0 and bsl == 0:
                        for jsl in range(NJ2):
                            emit_colsum(jsl)
                        pq2 = ps.tile([P, CT], F32, tag="ps")
                        for jt_ in range(CT):
                            nc.tensor.matmul(
                                pq2[:, jt_ : jt_ + 1],
                                crow[0:1, jt_ * P : (jt_ + 1) * P],
                                one1[:], start=True, stop=True)
                        nc.vector.reciprocal(rcol[:], pq2[:])
                    zt = zpool.tile([P, S2], F32, tag="zrow2")
                    nc.scalar.mul(zt[:], psz[:], rcol[:, jt : jt + 1])
                    nc.sync.dma_start(
                        z_d[jt * P : (jt + 1) * P, bs2], zt[:]
                    )

    nc.compile()
    return nc


def _host_prep(X, Wk, Wq, Wk0, Wq0, C):
    wkt = np.concatenate(
        [np.ascontiguousarray(Wk.T),
         Wk0.reshape(1, C),
         np.zeros((P - 1, C), np.float32)], axis=0
    ).astype(BF16NP)
    wqt = np.concatenate(
        [np.ascontiguousarray(Wq.T),
         Wq0.reshape(1, C),
         np.zeros((P - 1, C), np.float32)], axis=0
    ).astype(BF16NP)
    xb = np.ascontiguousarray(X).astype(BF16NP)
    return xb, wkt, wqt


_CACHE = {}


def kernel(X, Wk, Wq, Wk0, Wq0):
    global LAST_EXEC_NS, LAST_TRACE
    X = np.asarray(X, dtype=np.float32)
    Wk = np.asarray(Wk, dtype=np.float32)
    Wq = np.asarray(Wq, dtype=np.float32)
    Wk0 = np.asarray(Wk0, dtype=np.float32)
    Wq0 = np.asarray(Wq0, dtype=np.float32)
    N, C, B = X.shape
    assert N == N_CORES

    from concourse.bass_utils import run_bass_kernel_spmd

    with_bias = bool(np.any(Wk0) or np.any(Wq0))
    key = (C, B, with_bias)
    if key not in _CACHE:
        _CACHE[key] = build_program(C, B, with_bias)
    nc = _CACHE[key]

    xb, wkt, wqt = _host_prep(X, Wk, Wq, Wk0, Wq0, C)
    in_maps = [
        {"x": xb[n], "wkt": wkt, "wqt": wqt}
        for n in range(N)
    ]
    if TRACE:
        _install_ntff_shim()
    t0 = time.time()
    res = run_bass_kernel_spmd(
        nc, in_maps, core_ids=list(range(N_CORES)), trace=TRACE
    )
    wall_ns = int((time.time() - t0) * 1e9)
    LAST_EXEC_NS = (
        res.exec_time_ns if getattr(res, "exec_time_ns", None) else wall_ns
    )
    if getattr(res, "instructions_and_trace", None):
        LAST_TRACE = res.instructions_and_trace[1]
    out = np.stack([res.results[n]["z"] for n in range(N)], axis=0)
    return out.astype(np.float32)


if __name__ == "__main__":
    # small-scale self-test vs numpy
    C, B = 512, 512
    rng = np.random.default_rng(1)
    Xs = rng.standard_normal((N_CORES, C, B), dtype=np.float32)
    bound = float(np.sqrt(6.0 / (C + C)))
    Wks = rng.uniform(-bound, bound, (C, C)).astype(np.float32)
    Wqs = rng.uniform(-bound, bound, (C, C)).astype(np.float32)
    Wk0s = rng.standard_normal((C, 1)).astype(np.float32) * 0.01
    Wq0s = rng.standard_normal((C, 1)).astype(np.float32) * 0.01

    def ref(X, Wk, Wq, Wk0, Wq0):
        K = np.einsum("ij,njb->nib", Wk, X) + Wk0
        Q = np.einsum("ij,njb->nib", Wq, X) + Wq0
        DK2 = np.sum(K * K, axis=2)
        DQ2 = np.sum(Q * Q, axis=2)
        DQK = np.sqrt(np.maximum(DQ2[:, :, None] * DK2[:, None, :], 1e-12))
        Y = np.einsum("nib,njb->nij", Q, K) / DQK
        Y = Y - Y.max(axis=1, keepdims=True)
        E = np.exp(Y)
        SM = E / E.sum(axis=1, keepdims=True)
        return np.einsum("ncb,ncj->njb", X, SM)

    expected = ref(
        Xs.astype(np.float64), Wks.astype(np.float64),
        Wqs.astype(np.float64), Wk0s.astype(np.float64),
        Wq0s.astype(np.float64),
    )
    actual = kernel(Xs, Wks, Wqs, Wk0s, Wq0s)
    rel = np.linalg.norm(actual - expected) / np.linalg.norm(expected)
    print(f"small test relative error: {rel:.3e}")
    print(f"wall ns: {LAST_EXEC_NS}")
